# revision 1
# baseline (speedup 1.0000x reference)
"""GAT layer kernel for Trainium2, data-parallel over batch across 8 NeuronCores.

Reference computation (per batch b):
    Wh   = x @ W                                  [N, F]
    s_src = Wh @ a_w[:F];  s_dst = Wh @ a_w[F:]   [N]
    e    = s_src[:, None] + s_dst[None, :] + a_b  [N, N]
    exps = exp(leaky_relu(e, 0.2)) * A
    attn = exps / (exps.sum(axis=0) + 1e-7)       # softmax over dim i
    out  = attn @ Wh

Device strategy (per core = one batch):
  * The host prepares the full transposed score matrix with the mask folded
    in additively:
        AmT[j, i] = e[i, j] - C2 * (1 - A[i, j])        (C2 = 150)
    Masked entries carry an exponent shift of -150; after leaky_relu that is
    ~-30 and exp gives ~1e-13, which flushes to zero in fp16. Unmasked
    entries are bit-exact e values. This removes every on-device transpose
    and bias: tiles stream in already in [j, i] layout.
  * leaky_relu is split between ACT (Prelu, fast path) and DVE
    (one scalar_tensor_tensor: max(0.2*z, z)) to balance engines.
  * exp runs on ACT with fp32 output (16-bit outputs of Exp take a ~10x
    slower ucode path) + accum_out producing the softmax column sums.
  * fp32 -> fp16 happens via DVE tensor_copy (the only fast 16-bit writer).
  * Softmax division folds into Wh rows: whs = Wh * (1/(sums+eps)) [j].
  * Final matmul per i-tile: out += expsT[j, i-block].T @ whs[j] in fp16.
  * DMA is spread across the sync/scalar HWDGE queues and the gpsimd SWDGE
    queue; a single queue tops out near 66 GB/s which would dominate.
"""

import numpy as np

import concourse.bass as bass
import concourse.mybir as mybir
import concourse.tile as tile
from concourse import bacc
from concourse.bass_utils import run_bass_kernel_spmd

B, N, F = 8, 2048, 256
NT = N // 128          # 16 j-tiles
NJG = 4                # j-tile groups (softmax sums complete per group)
JPG = NT // NJG
NCH = 2                # i-chunks of 1024 per j-tile row
CHW = N // NCH
C2 = 150.0
EPS = 1e-7
NEG_SLOPE = 0.2
ACT_SHARE = 3          # of every 3 chunks, this many go to the ACT lrelu path (rest DVE)
import os
ABLATE = os.environ.get("GAT_ABLATE", "full")  # full|dma|elem|noout
ACT_SHARE = int(os.environ.get("GAT_ACT_SHARE", ACT_SHARE))
NJG = int(os.environ.get("GAT_NJG", NJG))
JPG = NT // NJG
DMA3 = os.environ.get("GAT_DMA3", "0") == "1"

f32 = mybir.dt.float32
f16 = mybir.dt.float16

AF = mybir.ActivationFunctionType
ALU = mybir.AluOpType


def build(nc, loop_n=None):
    amt_d = nc.declare_dram_parameter("amt", [N, N], f32, isOutput=False)
    xt_d = nc.declare_dram_parameter("xt", [F, N], f16, isOutput=False)
    w_d = nc.declare_dram_parameter("w16", [F, F], f16, isOutput=False)
    out_d = nc.declare_dram_parameter("out", [N, F], f32, isOutput=True)
    if loop_n == "dyn":
        nrep_d = nc.declare_dram_parameter("nrep", [1, 1], mybir.dt.int32, isOutput=False)

    dma_engines = None  # set inside context

    with tile.TileContext(nc) as tc:
        with (
            tc.tile_pool(name="const", bufs=1) as const,
            tc.tile_pool(name="xt", bufs=2) as xtp,
            tc.tile_pool(name="wh", bufs=NT) as whp,
            tc.tile_pool(name="whs", bufs=JPG + 1) as whsp,
            tc.tile_pool(name="expsT", bufs=NT) as expp,
            tc.tile_pool(name="zst", bufs=5) as zp,
            tc.tile_pool(name="tst", bufs=3) as tp_,
            tc.tile_pool(name="ust", bufs=3) as up,
            tc.tile_pool(name="sums", bufs=1) as sump,
            tc.tile_pool(name="outsb", bufs=NT) as outp,
            tc.tile_pool(name="mm1ps", bufs=2, space="PSUM") as mm1ps,
            tc.tile_pool(name="outps", bufs=3, space="PSUM") as outps,
        ):
            w16a = const.tile([128, F], f16)
            w16b = const.tile([128, F], f16)
            nc.sync.dma_start(w16a[:], w_d[0:128, :])
            nc.sync.dma_start(w16b[:], w_d[128:256, :])

            def body(_iv=None):
                xt0 = xtp.tile([128, N], f16, tag="xt")
                xt1 = xtp.tile([128, N], f16, tag="xt")
                nc.sync.dma_start(xt0[:], xt_d[0:128, :])
                nc.sync.dma_start(xt1[:], xt_d[128:256, :])

                # ---- Wh = x @ W, tiles [128 j, 256 o] fp32 ----
                wh = []
                for nt in range(NT if ABLATE not in ("dma", "elem") else 0):
                    ps = mm1ps.tile([128, F], f32)
                    sl = slice(nt * 128, (nt + 1) * 128)
                    nc.tensor.matmul(ps[:], xt0[:, sl], w16a[:], start=True, stop=False)
                    nc.tensor.matmul(ps[:], xt1[:, sl], w16b[:], start=False, stop=True)
                    t = whp.tile([128, F], f32, tag="wh")
                    nc.vector.tensor_copy(t[:], ps[:])
                    wh.append(t)

                sums_acc = sump.tile([128, NT, NCH], f32, tag="sa")
                sums_red = sump.tile([128, NT], f32, tag="sr")
                recip = sump.tile([128, NT], f32, tag="rc")
                expsT = [expp.tile([128, N], f16, tag="ex", name=f"expsT{j}")
                         for j in range(NT)]
                outsb = [outp.tile([128, F], f32, tag="ob", name=f"outsb{i}")
                         for i in range(NT)]

                chunk_idx = 0
                for jg in range(NJG):
                    for jl in range(JPG):
                        jt = jg * JPG + jl
                        for ch in range(NCH):
                            i0 = ch * CHW
                            z = zp.tile([128, CHW], f32, tag="z")
                            # two 256KB DMAs on different queues per strip
                            h = CHW // 2
                            eng0 = dma_engines[chunk_idx % 3]
                            eng1 = dma_engines[(chunk_idx + 1) % 3]
                            eng0.dma_start(
                                z[:, 0:h],
                                amt_d[jt * 128 : (jt + 1) * 128, i0 : i0 + h],
                            )
                            eng1.dma_start(
                                z[:, h:CHW],
                                amt_d[jt * 128 : (jt + 1) * 128, i0 + h : i0 + CHW],
                            )
                            if ABLATE == "dma":
                                chunk_idx += 1
                                continue
                            t = tp_.tile([128, CHW], f32, tag="t")
                            if chunk_idx % 3 < ACT_SHARE:
                                nc.scalar.activation(
                                    t[:], z[:], AF.Prelu,
                                    bias=0.0, scale=1.0, alpha=NEG_SLOPE,
                                )
                            else:
                                nc.vector.scalar_tensor_tensor(
                                    t[:], z[:], NEG_SLOPE, z[:],
                                    op0=ALU.mult, op1=ALU.max,
                                )
                            u = up.tile([128, CHW], f32, tag="u")
                            nc.scalar.activation(
                                u[:], t[:], AF.Exp, bias=0.0, scale=1.0,
                                accum_out=sums_acc[:, jt : jt + 1, ch : ch + 1],
                            )
                            nc.vector.tensor_copy(
                                expsT[jt][:, i0 : i0 + CHW], u[:]
                            )
                            chunk_idx += 1

                    if ABLATE in ("dma", "elem"):
                        continue
                    # ---- normalize: recip = 1/(sums+eps); whs = wh*recip ----
                    jsl = slice(jg * JPG, (jg + 1) * JPG)
                    nc.vector.tensor_reduce(
                        sums_red[:, jsl], sums_acc[:, jsl, :],
                        axis=mybir.AxisListType.X, op=ALU.add,
                    )
                    nc.vector.tensor_scalar(
                        sums_red[:, jsl], sums_red[:, jsl], EPS, None, op0=ALU.add
                    )
                    nc.vector.reciprocal(recip[:, jsl], sums_red[:, jsl])
                    whs = {}
                    for jl in range(JPG):
                        jt = jg * JPG + jl
                        w32 = tp_.tile([128, F], f32, tag="w32")
                        nc.scalar.activation(
                            w32[:], wh[jt][:], AF.Identity,
                            bias=0.0, scale=recip[:, jt : jt + 1],
                        )
                        ws16 = whsp.tile([128, F], f16, tag="whs")
                        nc.vector.tensor_copy(ws16[:], w32[:])
                        whs[jt] = ws16

                    # ---- partial output: out[i,:] += expsT^T @ whs ----
                    for it in range(NT):
                        po = outps.tile([128, F], f32)
                        for jl in range(JPG):
                            jt = jg * JPG + jl
                            nc.tensor.matmul(
                                po[:],
                                expsT[jt][:, it * 128 : (it + 1) * 128],
                                whs[jt][:],
                                start=(jl == 0), stop=(jl == JPG - 1),
                            )
                        if jg == 0:
                            nc.vector.tensor_copy(outsb[it][:], po[:])
                        else:
                            nc.vector.tensor_tensor(
                                outsb[it][:], po[:], outsb[it][:], op=ALU.add
                            )
                        if jg == NJG - 1:
                            nc.sync.dma_start(
                                out_d[it * 128 : (it + 1) * 128, :], outsb[it][:]
                            )

            dma_engines = [nc.sync, nc.gpsimd, nc.scalar] if DMA3 else [nc.sync, nc.gpsimd, nc.sync]
            if loop_n is None:
                body()
            elif loop_n == "dyn":
                nrep_t = const.tile([1, 1], mybir.dt.int32)
                nc.sync.dma_start(nrep_t[:], nrep_d[:])
                nval = nc.sync.value_load(nrep_t[:], min_val=1, max_val=1 << 20)
                with tc.For_i(0, nval, 1) as iv:
                    body(iv)
            else:
                with tc.For_i(0, loop_n, 1) as iv:
                    body(iv)

    nc.finalize()
    return nc


def _host_prep(A, x, W, a_w, a_b):
    """Per-core input maps from full inputs."""
    W64 = W.astype(np.float64)
    ha = W64 @ a_w[:F].astype(np.float64)
    hb = W64 @ a_w[F:].astype(np.float64)
    w16 = W.astype(np.float16)
    in_maps = []
    for b in range(B):
        xb = x[b]
        ssrc = (xb.astype(np.float64) @ ha).astype(np.float32)
        sdst = (xb.astype(np.float64) @ hb + float(a_b)).astype(np.float32)
        amt = (A[b].T - 1.0) * C2
        amt += ssrc[None, :]
        amt += sdst[:, None]
        xt16 = np.ascontiguousarray(xb.T).astype(np.float16)
        in_maps.append({"amt": np.ascontiguousarray(amt, dtype=np.float32),
                        "xt": xt16, "w16": w16})
    return in_maps


_NC_CACHE = {}


def _get_nc(loop_n=None):
    key = loop_n
    if key not in _NC_CACHE:
        _NC_CACHE[key] = build(bacc.Bacc(), loop_n=loop_n)
    return _NC_CACHE[key]


def kernel(A, x, W, a_w, a_b):
    A = np.asarray(A, dtype=np.float32)
    x = np.asarray(x, dtype=np.float32)
    W = np.asarray(W, dtype=np.float32)
    a_w = np.asarray(a_w, dtype=np.float32)
    a_b = np.float32(a_b)
    nc = _get_nc()
    in_maps = _host_prep(A, x, W, a_w, a_b)
    res = run_bass_kernel_spmd(nc, in_maps, list(range(B)))
    return np.stack([res.results[b]["out"] for b in range(B)], axis=0)

